# revision 85
# baseline (speedup 1.0000x reference)
"""Window attention (BaseWindowAttention) Trainium2 kernel.

Data-parallel over the 8 (b,l) slices, one NeuronCore each. Host prep:
transpose each slice to [c, tok] with tokens in window order, quantize x and
the q/k/v weights to fp8e4 in DoubleRow [p, 2, f] packing, and build fp8
hi/lo relative-position bias tables. Device datapaths:

- q/k projection: fp8 DoubleRow matmuls (x8 . wqk8*2^8), PSUM drained with a
  2^-8 scale to bf16 f-tiles. fp8 noise on q/k only perturbs attention
  logits, which softmax damps (measured ~1.2e-2 end-to-end vs 2e-2 budget).
- v projection: 3-pass compensated fp8 DoubleRow (xh.wh + xl.wh + xh.wl);
  xl = x - fp8(x) stored UNSCALED so it rides fp8 subnormals and all three
  products accumulate in PSUM at one scale (~0.2% v error).
- dots: bf16, seeded by a const fp8-DoubleRow bias matmul (id x bias-hi +
  id/64 x bias-lo rows, row-doubled tables so dpsA/dpsB keep uniform
  tile_position rows); exp reads the biased PSUM directly.
- softmax: ones-column in v gives sums row 64 of the mm2 PSUM; DVE
  reciprocal [1,512]; gpsimd partition_broadcast (Pool ISA op, no DRAM
  round trip); DVE normalize-multiply writes at-tiles, lane-shifted for odd
  heads (no alo/vlo SWDGE hops; vlo via Pool SBUF copy).
- oproj: bf16, bias via ones-row seed matmul + ACT copy (even tiles, and
  every tile of the final chunk so the drain never waits on DVE) or a
  broadcast bout tensor-add on DVE (odd tiles); bf16 output DMA.

Software pipeline: per-engine queues execute in program order; per iter one
unit (head x 8-window oct) emits dots-pair/exp, mm2 one unit behind,
normalize NDELAY units behind, oproj at OP0/OP1, stage-1 of chunk c+1
spread LEAD iters behind just-in-time; chunk-0 x loads issue ahead of the
other constants; PSUM->SBUF drains split ACT/DVE per f-tile (chunk-0 leans
DVE while it has no norm work yet). PSUM: psA 3 + psD 2 + psM 3 = 8 banks
exactly (cost-model-tuned; see OP0/OP1/NDELAY/LEAD).

Landmines (bisected on trn2): column tile_position crashes the device;
mixing tile_position rows within one PSUM tile crashes the device; the Pool
engine cannot touch PSUM (compile fails); DMA cannot read PSUM; HWDGE/SWDGE
support one sync wait.

Self-contained: shapes hardcoded, no sibling imports.
"""
import numpy as np

import concourse.mybir as mybir
import concourse.tile as tile
from concourse import bacc
from concourse.bass_utils import run_bass_kernel_spmd

F32 = mybir.dt.float32
F32R = mybir.dt.float32r
BF16 = mybir.dt.bfloat16
FP8 = mybir.dt.float8e4
DR = mybir.MatmulPerfMode.DoubleRow
QK_SHIFT = 8                          # wqk pre-scale 2**8 for fp8 range

B, L, H, W, C = 2, 4, 64, 64, 512
HEADS, CH, WS = 8, 64, 8
WTOK = WS * WS                        # 64 tokens per window
TOK = H * W                           # 4096 tokens per slice
INNER = HEADS * CH                    # 512
SCALE = CH ** -0.5                    # 0.125
CHUNK = 1024                          # tokens per pipeline chunk (16 windows)
NCHUNK = TOK // CHUNK                 # 4
NUNITS = 16                           # attention units per chunk (8 heads x 2)
NITER = NUNITS + 8                    # pipeline drain iterations
NDELAY = 2                            # norm lag units (recip+bcast latency)
OP0, OP1 = 10, 18                     # oproj emission offsets per chunk
LEAD = 10                             # stage-1 lead (iters) for chunk c+1
SPREAD_SCALE = 1.0                    # stretch factor for stage-1 spread
NCORES = 8

_NC_CACHE = None


def build_nc():
    nc = bacc.Bacc()

    # fp8 copies of x packed for DoubleRow: index [p, j*2+i, t] holds
    # channel c = j*256 + i*128 + p
    xt8_d = nc.dram_tensor("xt8", [128, 4, TOK], FP8, kind="ExternalInput")
    # xl8 = (x - fp8(x)) stored unscaled (rides fp8 subnormals) so all three
    # compensation products accumulate in PSUM at one scale
    xl8_d = nc.dram_tensor("xl8", [128, 4, TOK], FP8, kind="ExternalInput")
    wqk8_d = nc.dram_tensor("wqk8", [128, 4, 2 * INNER], FP8, kind="ExternalInput")
    # wv8[:, :, 0:INNER] = fp8(wv*256) hi, [:, :, INNER:] = residual lo
    wv8_d = nc.dram_tensor("wv8", [128, 4, 2 * INNER], FP8, kind="ExternalInput")
    wout_d = nc.dram_tensor("wout", [INNER, C], F32, kind="ExternalInput")
    bout_d = nc.dram_tensor("bout", [C], F32, kind="ExternalInput")
    # fp8 DoubleRow bias tables (row-doubled so dpsA uses partitions 0:64
    # and dpsB 64:128 -- keeps tile_position rows unmixed per PSUM tile):
    # biasw[j or j+64, 0, k] = fp8(8*bias[j,k]); [.., 1, k] = fp8(res*64)
    biasw_d = nc.dram_tensor("biasw", [128, 2, WTOK], FP8, kind="ExternalInput")
    # idrep[p or p+64, 0, c] = (c%64==p); [.., 1, c] = (c%64==p)/64
    idrep_d = nc.dram_tensor("idrep", [128, 2, 8 * WTOK], FP8, kind="ExternalInput")
    out_d = nc.dram_tensor("out", [TOK, C], BF16, kind="ExternalOutput")

    with tile.TileContext(nc) as tc:
        with (
            tc.tile_pool(name="const", bufs=1) as cpool,
            tc.tile_pool(name="sb", bufs=2) as sb,
            tc.tile_pool(name="attS", bufs=3) as attS,
            tc.tile_pool(name="attL", bufs=7) as attL,
            tc.tile_pool(name="psA", bufs=3, space="PSUM") as psA,
            tc.tile_pool(name="psD", bufs=1, space="PSUM") as psD,
            tc.tile_pool(name="psM", bufs=3, space="PSUM") as psM,
        ):
            # ---- constants (spread across SP/ACT/Pool queues so the first
            # stage-1 matmul's inputs land fast) ----
            wqk_sb = cpool.tile([128, 4, 2 * INNER], FP8, tag="wqk")
            engs = [nc.sync, nc.scalar, nc.gpsimd, nc.gpsimd]
            wengs = [nc.sync, nc.scalar, nc.sync, nc.scalar]
            for jt in range(2):
                for fh in range(2):
                    wengs[jt * 2 + fh].dma_start(
                        out=wqk_sb[:, jt * 2 : (jt + 1) * 2,
                                   fh * 512 : (fh + 1) * 512],
                        in_=wqk8_d.ap()[:, jt * 2 : (jt + 1) * 2,
                                        fh * 512 : (fh + 1) * 512],
                    )
            def load_xt(ch):
                t0 = ch * CHUNK
                xt8_sb = sb.tile([128, 4, CHUNK], FP8, tag="xt8", name="xt8")
                for jt in range(2):
                    eng = nc.gpsimd if jt == 0 else nc.sync
                    eng.dma_start(
                        out=xt8_sb[:, jt * 2 : (jt + 1) * 2, :],
                        in_=xt8_d.ap()[:, jt * 2 : (jt + 1) * 2, t0 : t0 + CHUNK],
                    )
                xl8_sb = sb.tile([128, 4, CHUNK], FP8, tag="xl8", name="xl8")
                for jt in range(2):
                    eng = nc.sync if jt == 0 else nc.gpsimd
                    eng.dma_start(
                        out=xl8_sb[:, jt * 2 : (jt + 1) * 2, :],
                        in_=xl8_d.ap()[:, jt * 2 : (jt + 1) * 2, t0 : t0 + CHUNK],
                    )
                return xl8_sb, xt8_sb

            # chunk-0 activations issue ahead of the remaining constants so
            # the first stage-1 groups aren't queued behind them
            xt_pair0 = load_xt(0)

            wv_sb = cpool.tile([128, 4, 2 * INNER], FP8, tag="wv")
            for jt in range(2):
                engs[2 + jt].dma_start(
                    out=wv_sb[:, jt * 2 : (jt + 1) * 2, :],
                    in_=wv8_d.ap()[:, jt * 2 : (jt + 1) * 2, :],
                )
            wout_sb = cpool.tile([128, 4, C], BF16, tag="wout")
            nc.gpsimd.dma_start(
                out=wout_sb[:], in_=wout_d.ap().rearrange("(kt p) f -> p kt f", p=128)
            )
            biasw_sb = cpool.tile([128, 2, WTOK], FP8, tag="biasw")
            nc.sync.dma_start(out=biasw_sb[:], in_=biasw_d.ap())
            idrep_sb = cpool.tile([128, 2, 8 * WTOK], FP8, tag="idrep")
            nc.sync.dma_start(out=idrep_sb[:], in_=idrep_d.ap())
            bout_sb = cpool.tile([1, C], F32, tag="bout")
            nc.gpsimd.dma_start(out=bout_sb[:], in_=bout_d.ap().unsqueeze(0))
            boutb_sb = cpool.tile([128, C], F32, tag="boutb")
            nc.gpsimd.partition_broadcast(boutb_sb[:], bout_sb[:], 128)
            bout16_sb = cpool.tile([1, C], BF16, tag="bout16")
            nc.gpsimd.tensor_copy(out=bout16_sb[:], in_=bout_sb[:])
            onesr_sb = cpool.tile([1, CHUNK], BF16, tag="onesr")
            nc.gpsimd.memset(onesr_sb[:], 1.0)

            def stage1_groups(xt_pair, ch=1):
                """Return (emitters, results): 24 matmul-group thunks building
                qkT f-tiles and v tiles for one chunk."""
                xl8_sb, xt8_sb = xt_pair
                qk_sb = [
                    sb.tile([128, CHUNK], BF16, tag=f"qk{ft}", name=f"qk{ft}")
                    for ft in range(8)
                ]
                v_sb = [
                    sb.tile([128, HEADS * 65], BF16, tag=f"v{tt}", name=f"v{tt}")
                    for tt in range(CHUNK // 128)
                ]
                vlo_sb = [
                    sb.tile([64, HEADS * 65], BF16, tag=f"vlo{tt}", name=f"vlo{tt}")
                    for tt in range(CHUNK // 128)
                ]
                emitters = []

                def qk_group(ft, th):
                    def emit():
                        ps = psA.tile([128, 512], F32, tag="psA", name="psA")
                        for jt in range(2):
                            nc.tensor.matmul(
                                ps[:],
                                wqk_sb[:, jt * 2 : (jt + 1) * 2,
                                       ft * 128 : (ft + 1) * 128],
                                xt8_sb[:, jt * 2 : (jt + 1) * 2,
                                       th * 512 : (th + 1) * 512],
                                start=(jt == 0),
                                stop=(jt == 1),
                                perf_mode=DR,
                            )
                        dst = qk_sb[ft][:, th * 512 : (th + 1) * 512]
                        if (ft % 2 == 1) if ch == 0 else (
                                ft == 7 or (ft == 5 and th == 0)):
                            nc.vector.tensor_scalar_mul(
                                dst, ps[:], 2.0 ** -QK_SHIFT
                            )
                        else:
                            nc.scalar.mul(dst, ps[:], 2.0 ** -QK_SHIFT)

                    return emit

                def v_group(tt):
                    def emit():
                        ps = psA.tile([128, 512], F32, tag="psA", name="psA")
                        passes = [
                            (xt8_sb, 0),   # x_hi . wv_hi
                            (xl8_sb, 0),   # x_lo . wv_hi
                            (xt8_sb, 512),  # x_hi . wv_lo
                        ]
                        for pi, (xs, w0) in enumerate(passes):
                            for jt in range(2):
                                nc.tensor.matmul(
                                    ps[:],
                                    xs[:, jt * 2 : (jt + 1) * 2,
                                       tt * 128 : (tt + 1) * 128],
                                    wv_sb[:, jt * 2 : (jt + 1) * 2,
                                          w0 : w0 + 512],
                                    start=(pi == 0 and jt == 0),
                                    stop=(pi == 2 and jt == 1),
                                    perf_mode=DR,
                                )
                        vv = v_sb[tt][:].rearrange("p (m c) -> p m c", c=65)
                        if ch == 0 and tt % 2 == 1:
                            nc.vector.tensor_scalar_mul(
                                vv[:, :, 0:64],
                                ps[:].rearrange("p (m c) -> p m c", c=64),
                                2.0 ** -QK_SHIFT,
                            )
                        else:
                            nc.scalar.mul(
                                vv[:, :, 0:64],
                                ps[:].rearrange("p (m c) -> p m c", c=64),
                                2.0 ** -QK_SHIFT,
                            )
                        nc.gpsimd.memset(vv[:, :, 64:65], 1.0)
                        # odd window rows down to 0..63 for mm2 (lane-shifted
                        # engine copy instead of a SWDGE round trip)
                        nc.gpsimd.tensor_copy(
                            out=vlo_sb[tt][:], in_=v_sb[tt][64:128, :]
                        )

                    return emit

                for ft in range(8):
                    for th in range(CHUNK // 512):
                        emitters.append(qk_group(ft, th))
                for tt in range(CHUNK // 128):
                    emitters.append(v_group(tt))
                return emitters, (qk_sb, v_sb, vlo_sb)

            # ---- one continuous software pipeline across all chunks ----
            chunk_tiles = {}
            chunk_at = {}
            state = {}  # global unit -> dict of tiles for delayed stages

            def get_at(ch):
                if ch not in chunk_at:
                    chunk_at[ch] = [
                        sb.tile([128, CHUNK], BF16, tag=f"at{kt}", name=f"at{kt}")
                        for kt in range(4)
                    ]
                return chunk_at[ch]

            def emit_front_pair(g):
                # dots for the even/odd head pair (g, g+1), emitted
                # interleaved so the row-0 and row-64 matmuls sit adjacent in
                # the PE queue and run concurrently on the 32-row sub-arrays
                # (separate PSUM tiles per unit — same-tile row mixing is a
                # device crash). Then bias + exp per unit.
                ch, u = divmod(g, NUNITS)
                qk_sb, v_sb, vlo_sb = chunk_tiles[ch]
                oct_, m = divmod(u, 8)
                qf = qk_sb[m // 2]
                kf = qk_sb[4 + m // 2]
                dpsA = psD.tile([64, 512], F32, tag="psDA", name="psDA")
                dpsB = psD.tile([64, 512], F32, tag="psDB", name="psDB")
                # relative-position bias seeds the PSUM accumulators via a
                # const fp8 DoubleRow matmul (hi + lo/64 rows), then the 16
                # window dots accumulate on top.
                for dps, hrow in ((dpsA, 0), (dpsB, 64)):
                    nc.tensor.matmul(
                        dps[:],
                        biasw_sb[hrow : hrow + 64, :, :],
                        idrep_sb[hrow : hrow + 64, :, :],
                        start=True,
                        stop=False,
                        perf_mode=DR,
                        tile_position=(hrow, 0),
                        skip_group_check=True,
                    )
                for nl in range(8):
                    ncol = (oct_ * 8 + nl) * 64
                    for dps, hrow in ((dpsA, 0), (dpsB, 64)):
                        nc.tensor.matmul(
                            dps[:, nl * 64 : (nl + 1) * 64],
                            kf[hrow : hrow + 64, ncol : ncol + 64],
                            qf[hrow : hrow + 64, ncol : ncol + 64],
                            start=False,
                            stop=True,
                            tile_position=(hrow, 0),
                            skip_group_check=True,
                        )
                for gg, dps in ((g, dpsA), (g + 1, dpsB)):
                    e_t = attL.tile([64, 512], BF16, tag="e", name="e")
                    nc.scalar.activation(
                        e_t[:], dps[:], mybir.ActivationFunctionType.Exp,
                        scale=SCALE,
                    )
                    mm = m + (gg - g)
                    state[gg] = {"e": e_t, "m": mm, "oct": oct_, "ch": ch}

            def emit_mid(g):
                # mm2 (+ones column -> sums row 64) + recip round trip
                st = state[g]
                m, oct_, e_t, ch = st["m"], st["oct"], st["e"], st["ch"]
                _, v_sb, vlo_sb = chunk_tiles[ch]
                ops = psM.tile([65, 512], F32, tag="psM", name="psM")
                for nl in range(8):
                    tt = oct_ * 4 + nl // 2
                    if nl % 2 == 0:
                        lhsT = v_sb[tt][0:64, m * 65 : (m + 1) * 65]
                    else:
                        lhsT = vlo_sb[tt][:, m * 65 : (m + 1) * 65]
                    nc.tensor.matmul(
                        ops[:, nl * 64 : (nl + 1) * 64],
                        lhsT,
                        e_t[:, nl * 64 : (nl + 1) * 64],
                        start=True,
                        stop=True,
                    )
                r_t = attS.tile([1, 512], F32, tag="r", name="r")
                nc.vector.reciprocal(r_t[:], ops[64:65, :])
                norm = attL.tile([64, 512], F32, tag="norm", name="norm")
                nc.gpsimd.partition_broadcast(norm[:], r_t[:], 64)
                st["norm"] = norm
                st["ops"] = ops

            def emit_norm(g):
                # normalize + AT write
                st = state.pop(g)
                m, oct_, ch = st["m"], st["oct"], st["ch"]
                at_sb = get_at(ch)
                kt = m // 2
                prow = (m % 2) * 64
                nc.vector.tensor_tensor(
                    at_sb[kt][prow : prow + 64, oct_ * 512 : (oct_ + 1) * 512],
                    st["ops"][0:64, :],
                    st["norm"][:],
                    mybir.AluOpType.mult,
                )

            def emit_oproj(ch, tt):
                at_sb = chunk_at[ch]
                ps = psA.tile([128, 512], F32, tag="psA", name="psA")
                o_t = sb.tile([128, C], BF16, tag="o", name="o")
                on_act = tt % 4 != 3
                if on_act:
                    nc.tensor.matmul(
                        ps[:],
                        onesr_sb[:, tt * 128 : (tt + 1) * 128],
                        bout16_sb[:],
                        start=True,
                        stop=False,
                    )
                for kt in range(4):
                    nc.tensor.matmul(
                        ps[:],
                        at_sb[kt][:, tt * 128 : (tt + 1) * 128],
                        wout_sb[:, kt, :],
                        start=(kt == 0) and not on_act,
                        stop=(kt == 3),
                    )
                if on_act:
                    nc.scalar.copy(o_t[:], ps[:])
                else:
                    nc.vector.tensor_add(o_t[:], ps[:], boutb_sb[:])
                nc.sync.dma_start(
                    out=out_d.ap()[
                        ch * CHUNK + tt * 128 : ch * CHUNK + (tt + 1) * 128, :
                    ],
                    in_=o_t[:],
                )

            # ---- stage-1 work scheduled just-in-time ----
            # sched[i] = thunks to run at global iteration i. Chunk c's 24
            # groups run during chunk c-1's unit iterations (i in [base,
            # base+16)), reordered so the groups each unit depends on first
            # (qk th=0 + v 0..3 before oct 0; th=1 + v 4..7 before oct 1).
            NG = NCHUNK * NUNITS  # 64 global units
            PRE = 3               # pre-iterations for chunk 0's stage 1
            sched = {}

            def at_iter(i, fn):
                sched.setdefault(i, []).append(fn)

            # emitters list layout from stage1_groups: qk idx = ft*2+th,
            # v idx = 16+tt. Criticality order:
            group_order = []
            for th in range(2):
                for ft in (0, 4, 1, 5, 2, 6, 3, 7):
                    group_order.append(ft * 2 + th)
                group_order.extend(16 + th * 4 + tt for tt in range(4))

            # iteration offsets (from chunk base) for the 24 ordered groups;
            # group j must complete before the unit that reads it: th0 qk by
            # +16, v 0..3 by +17, th1 qk by +24, v 4..7 by +25 (chunk c units
            # run at global iters [c*16, c*16+16) = [base+16, base+40)).
            spread = [0, 0, 1, 1, 2, 2, 4, 4, 3, 3, 5, 5,
                      8, 8, 9, 9, 10, 10, 11, 11, 12, 12, 13, 13]
            # chunk 0 has no predecessor: compress the critical prefix
            spread0 = [-2, -2, -1, -1, 1, 1, 2, 2, -1, -1, 0, 0,
                       3, 3, 4, 4, 5, 5, 6, 6, 7, 7, 8, 8]

            def schedule_chunk(ch):
                emitters, tiles = stage1_groups(
                    xt_pair0 if ch == 0 else load_xt(ch), ch
                )
                chunk_tiles[ch] = tiles
                if ch == 0:
                    offs, base = spread0, 0
                else:
                    offs = [int(round(o * SPREAD_SCALE)) + LEAD for o in spread]
                    base = (ch - 1) * NUNITS
                for j, gi in enumerate(group_order):
                    at_iter(base + offs[j], emitters[gi])

            for i in range(-PRE, NG + 24):
                if i == -PRE:
                    schedule_chunk(0)
                for ch in range(1, NCHUNK):
                    if i == (ch - 1) * NUNITS:
                        schedule_chunk(ch)
                # last-oct norms run at a tighter lag to shorten the drain
                g_n = i - 1 - NDELAY
                g_last = NG - NDELAY
                if 0 <= g_n < min(NG, g_last):
                    emit_norm(g_n)
                if NDELAY > 1 and g_n == g_last:
                    for g_x in range(g_last, NG):
                        emit_norm(g_x)
                if 0 <= i - 1 < NG:
                    emit_mid(i - 1)
                if 0 <= i < NG and i % 2 == 0:
                    emit_front_pair(i)
                for ch in range(NCHUNK):
                    cb = ch * NUNITS
                    if cb + OP0 <= i <= cb + OP0 + 3:
                        emit_oproj(ch, i - cb - OP0)
                    if ch == NCHUNK - 1:
                        if cb + OP1 - 1 <= i <= cb + OP1:
                            tt = (i - cb - OP1 + 1) * 2 + 4
                            emit_oproj(ch, tt)
                            emit_oproj(ch, tt + 1)
                    elif cb + OP1 <= i <= cb + OP1 + 3:
                        emit_oproj(ch, i - cb - OP1 + 4)
                for fn in sched.get(i, []):
                    fn()

    nc.finalize()
    return nc


def _get_nc():
    global _NC_CACHE
    if _NC_CACHE is None:
        _NC_CACHE = build_nc()
    return _NC_CACHE


def _bias_tables(pos_emb: np.ndarray):
    """fp8 DoubleRow bias matmul constants: (biasw [128,2,64], idrep
    [128,2,512]).  out[k,q] = sum_j biasw[j,i,k]*idrep[j,i,q] = 8*bias[q,k]
    with hi + lo/64 compensation; rows doubled for dpsA/dpsB positions."""
    import ml_dtypes

    F8 = ml_dtypes.float8_e4m3
    idx = np.array([[i, j] for i in range(WS) for j in range(WS)])
    rel = idx[None, :, :] - idx[:, None, :] + WS - 1
    bias = pos_emb[rel[:, :, 0], rel[:, :, 1]].astype(np.float32)  # [q, k]
    b8 = bias * 8.0
    bhi = b8.astype(F8)
    blo = ((b8 - bhi.astype(np.float32)) * 64.0).astype(F8)
    bw = np.zeros((64, 2, WTOK), F8)
    bw[:, 0, :] = bhi
    bw[:, 1, :] = blo
    biasw = np.concatenate([bw, bw], axis=0)            # [128, 2, 64]
    ids = np.zeros((64, 2, 8 * WTOK), np.float32)
    cols = np.arange(8 * WTOK)
    ids[cols % 64, 0, cols] = 1.0
    ids[cols % 64, 1, cols] = 1.0 / 64.0
    ids = ids.astype(F8)
    idrep = np.concatenate([ids, ids], axis=0)          # [128, 2, 512]
    return biasw, idrep


def host_prep(x, w_qkv, pos_emb, w_out, b_out):
    """Shard + lay out the inputs: one in_map per core."""
    x = np.ascontiguousarray(np.asarray(x, dtype=np.float32))
    w_qkv = np.asarray(w_qkv, dtype=np.float32)
    pos_emb = np.asarray(pos_emb, dtype=np.float32)
    w_out = np.ascontiguousarray(np.asarray(w_out, dtype=np.float32))
    b_out = np.ascontiguousarray(np.asarray(b_out, dtype=np.float32))

    import ml_dtypes

    F8 = ml_dtypes.float8_e4m3
    nh = H // WS
    # [slice, c, tok'] with tok' in window order (nh, nw, wsh, wsw)
    xt = x.reshape(B * L, nh, WS, nh, WS, C).transpose(0, 5, 1, 3, 2, 4)
    xt = np.ascontiguousarray(xt.reshape(B * L, C, TOK))

    def pack_dr(a, last):
        # [BL?, 512, F] -> [BL?, 128, 4, F] with c = j*256 + i*128 + p
        lead = a.shape[:-2]
        return np.ascontiguousarray(
            a.reshape(*lead, 2, 2, 128, last)
            .swapaxes(-2, -4).swapaxes(-2, -3)
            .reshape(*lead, 128, 4, last)
        )

    xt8 = pack_dr(xt, TOK).astype(F8)
    xl8 = (pack_dr(xt, TOK) - xt8.astype(np.float32)).astype(F8)

    wqk = np.ascontiguousarray(w_qkv[:, : 2 * INNER])
    wqk8 = pack_dr(wqk * 2.0 ** QK_SHIFT, 2 * INNER).astype(F8)
    wv = np.ascontiguousarray(w_qkv[:, 2 * INNER :]) * 2.0 ** QK_SHIFT
    wvh = pack_dr(wv, INNER).astype(F8)
    wvl = (pack_dr(wv, INNER) - wvh.astype(np.float32)).astype(F8)
    wv8 = np.concatenate([wvh, wvl], axis=-1)   # [128, 4, 2*INNER]
    biasw, idrep = _bias_tables(pos_emb)
    return [
        {
            "xt8": xt8[s],
            "xl8": xl8[s],
            "wqk8": wqk8,
            "wv8": wv8,
            "wout": w_out,
            "bout": b_out,
            "biasw": biasw,
            "idrep": idrep,
        }
        for s in range(NCORES)
    ]


def host_post(out_slices):
    """[NCORES x (tok', c)] window-ordered -> [b, l, h, w, c]."""
    nh = H // WS
    out = np.stack([np.asarray(o) for o in out_slices])
    out = out.reshape(B * L, nh, nh, WS, WS, C).transpose(0, 1, 3, 2, 4, 5)
    return np.ascontiguousarray(out.reshape(B, L, H, W, C), dtype=np.float32)


def kernel(x, w_qkv, pos_emb, w_out, b_out):
    in_maps = host_prep(x, w_qkv, pos_emb, w_out, b_out)
    nc = _get_nc()
    res = run_bass_kernel_spmd(nc, in_maps, list(range(NCORES)))
    return host_post([res.results[s]["out"] for s in range(NCORES)])

